# revision 4
# baseline (speedup 1.0000x reference)
"""Trainium2 Bass kernel: f32 stream + tensor_reduce block-max + top-1
gather resolve (exact). bufs=3 (vs 2) hides the ~5 us/chunk semaphore dead time between a
chunk's reduce and the DMA ring reusing its buffer; measured ~410 us
median sustained (8 cores concurrent, ~359 GB/s/core shared-HBM floor
~365 us) vs ~436-450 us for the 2-buffer baseline.

Per-core (1024 rows):
  1. Stream 128-row tiles as f32 chunks on the HWDGE ring(s).
  2. tensor_reduce(max) over 128-wide blocks -> blockmax [128, 250]
     (real DVE runs this ~8 elem/lane/cycle; ~31 us/core total).
  3. vector.max + max_index -> winning block id (first-wins, exact).
  4. Indirect-DMA gather of the winning 128-wide block in f32.
  5. max_index in-block -> offset; final = block*128 + offset.
Tail (4-5) for tile t runs one tile behind the stream.
"""

import numpy as np

P = 128
V = 32000
B = 128
N_CORES = 8
ROWS_PER_CORE = 16 * 512 // N_CORES  # 1024

_cache = {}


def _build(rows, repeat=1, chunk=16000, bufs=3, b=B, rings=1, defer_tail=True):
    import concourse.bass as bass
    import concourse.bacc as bacc
    import concourse.mybir as mybir
    from concourse.tile import TileContext, add_dep_helper

    f32 = mybir.dt.float32
    i32 = mybir.dt.int32
    u32 = mybir.dt.uint32
    Alu = mybir.AluOpType

    nch = V // chunk
    cb = chunk // b
    nb = V // b
    assert chunk * nch == V and b * cb == chunk and b * nb == V

    nc = bacc.Bacc(trn_type="TRN2", debug=False)
    x = nc.dram_tensor("x", [rows, V], f32, kind="ExternalInput")
    y = nc.dram_tensor("y", [rows, 1], f32, kind="ExternalOutput")
    x_ap = x.ap()
    x_blocks = x_ap.rearrange("r (n b) -> (r n) b", b=b)
    n_tiles = rows // P

    with TileContext(nc) as tc:
        with (
            tc.tile_pool(name="data", bufs=bufs) as dpool,
            tc.tile_pool(name="small", bufs=3) as spool,
            tc.tile_pool(name="cst", bufs=1) as cpool,
        ):
            rowbase = cpool.tile([P, 1], i32)
            nc.gpsimd.iota(rowbase[:], [[1, 1]], base=0, channel_multiplier=nb)

            def tail(t, top8, blk8, gath, after=None):
                inb8 = spool.tile([P, 8], u32, tag="inb8")
                mi = nc.vector.max_index(
                    out=inb8[:], in_max=top8[:], in_values=gath[:]
                )
                if after is not None:
                    add_dep_helper(mi.ins, after.ins, sync=False,
                                   reason="tail after current tile reduces")
                fblk = spool.tile([P, 1], f32, tag="fblk")
                finb = spool.tile([P, 1], f32, tag="finb")
                nc.vector.tensor_copy(out=fblk[:], in_=blk8[:, 0:1])
                nc.vector.tensor_copy(out=finb[:], in_=inb8[:, 0:1])
                res = spool.tile([P, 1], f32, tag="res")
                nc.vector.scalar_tensor_tensor(
                    out=res[:], in0=fblk[:], scalar=float(b), in1=finb[:],
                    op0=Alu.mult, op1=Alu.add,
                )
                nc.scalar.dma_start(out=y.ap()[t * P:(t + 1) * P, :], in_=res[:])

            pending = []
            ci_global = 0
            for rep in range(repeat):
                for t in range(n_tiles):
                    blockmax = spool.tile([P, nb], f32, tag="blockmax")
                    last_reduce = None
                    for ci in range(nch):
                        col = ci * chunk
                        ch = dpool.tile([P, chunk], f32, tag="chunk")
                        eng = nc.sync if (rings == 1 or ci_global % 2 == 0) else nc.scalar
                        ci_global += 1
                        eng.dma_start(
                            out=ch[:],
                            in_=x_ap[t * P:(t + 1) * P, col:col + chunk],
                        )
                        last_reduce = nc.vector.tensor_reduce(
                            out=blockmax[:, col // b:(col + chunk) // b],
                            in_=ch[:].rearrange("p (n b) -> p n b", b=b),
                            axis=mybir.AxisListType.X,
                            op=Alu.max,
                        )

                    top8 = spool.tile([P, 8], f32, tag="top8")
                    blk8 = spool.tile([P, 8], u32, tag="blk8")
                    gath = spool.tile([P, b], f32, tag="gath")
                    gidx = spool.tile([P, 1], i32, tag="gidx")
                    nc.vector.max(out=top8[:], in_=blockmax[:])
                    nc.vector.max_index(
                        out=blk8[:], in_max=top8[:], in_values=blockmax[:]
                    )
                    nc.vector.tensor_tensor(
                        out=gidx[:], in0=rowbase[:],
                        in1=blk8[:, 0:1].bitcast(i32), op=Alu.add,
                    )
                    nc.gpsimd.indirect_dma_start(
                        out=gath[:],
                        out_offset=None,
                        in_=x_blocks,
                        in_offset=bass.IndirectOffsetOnAxis(ap=gidx[:, 0:1], axis=0),
                        element_offset=t * P * V,
                    )
                    if defer_tail:
                        pending.append((t, top8, blk8, gath))
                        if len(pending) > 1:
                            tail(*pending.pop(0), after=last_reduce)
                    else:
                        tail(t, top8, blk8, gath)

                for args in pending:
                    tail(*args)
                pending = []
    nc.compile()
    return nc


def get_nc(rows=ROWS_PER_CORE, repeat=1, **kw):
    key = (rows, repeat, tuple(sorted(kw.items())))
    if key not in _cache:
        _cache[key] = _build(rows, repeat, **kw)
    return _cache[key]


def kernel(output: np.ndarray) -> np.ndarray:
    """Full-input entry point: (16, 512, 32000) f32 -> (16, 512, 1) f32."""
    from concourse.bass_utils import run_bass_kernel_spmd

    n, d, v = output.shape
    assert (n, d, v) == (16, 512, V), (n, d, v)
    x = np.ascontiguousarray(output, dtype=np.float32).reshape(
        N_CORES, ROWS_PER_CORE, V
    )
    nc = get_nc(ROWS_PER_CORE)
    in_maps = [{"x": x[c]} for c in range(N_CORES)]
    res = run_bass_kernel_spmd(nc, in_maps, core_ids=list(range(N_CORES)))
    out = np.stack([res.results[c]["y"] for c in range(N_CORES)], axis=0)
    return out.reshape(n, d, 1).astype(np.float32)
